# revision 10
# baseline (speedup 1.0000x reference)
"""Bass/Trainium2 kernel for nn_DecorrelationGradient.

Reference computation (KAPPA = 0.5):
    out = (1-k)*(gram - diag_ms) + k*(diag_ms - 1)
        = 0.5 * (X^T X / N) - 0.5          (diag terms cancel algebraically)

with X = x.reshape(N, d), N = 8*2048 = 16384, d = 768.

Strategy (data-parallel over the sample axis, 8 cores):
  - core c gets x[c] : [2048, 768] f32
  - computes the upper-triangle blocks of its partial Gram P_c = x_c^T x_c
    on the PE (float32r matmuls, PSUM accumulation over 16 k-tiles)
  - fused scale+bias on the PSUM->SBUF copy:  t = P_c * (0.5/N) - 0.5/8
  - packed triangle [128, 2688] f32 -> DRAM, ReduceScatter(add) over 8 cores
  - each core outputs its 16-partition-row slice of the reduced packed
    triangle; the host concatenates the slices and unpacks the symmetric
    matrix (pure indexing, no arithmetic).
"""

import numpy as np

import concourse.bacc as bacc
import concourse.bass as bass  # noqa: F401  (AP helpers)
import concourse.tile as tile
from concourse import mybir
from concourse.bass_utils import run_bass_kernel_spmd

P = 128
D = 768
NSHARD = 2048          # samples per core
KT = NSHARD // P       # 16 k-tiles
NB = D // P            # 6 row/col blocks
NCORES = 8
NTOT = 8 * 2048
SCALE = 0.5 / NTOT     # 2**-15, exact
BIAS = -0.5 / NCORES   # -0.0625, exact; RS adds 8 copies -> -0.5

# packed upper-triangle blocks (i, j) with j >= i, row-major in i
TRI_BLOCKS = [(i, j) for i in range(NB) for j in range(i, NB)]
NTRI = len(TRI_BLOCKS)          # 21
TRI_W = NTRI * P                # 2688 packed columns
ROWS_PER_CORE = P // NCORES     # 16 partition rows of the packed triangle


def _split_free(width):
    """Split a moving free-dim into chunks <= 512 (one PSUM bank per matmul)."""
    out = []
    s = 0
    while s < width:
        w = min(512, width - s)
        out.append((s, s + w))
        s += w
    return out


def _build():
    nc = bacc.Bacc(num_devices=NCORES)

    x_sh = nc.dram_tensor(
        "x_shard", [NSHARD, D], mybir.dt.float32, kind="ExternalInput"
    )
    out_sh = nc.dram_tensor(
        "out_shard", [ROWS_PER_CORE, TRI_W], mybir.dt.float32, kind="ExternalOutput"
    )

    f32 = mybir.dt.float32
    bf16 = mybir.dt.bfloat16

    with tile.TileContext(nc) as tc:
        with (
            tc.tile_pool(name="xp", bufs=KT) as xpool,
            tc.tile_pool(name="ps", bufs=1, space="PSUM") as pspool,
            tc.tile_pool(name="acc", bufs=1) as accpool,
            tc.tile_pool(name="dram", bufs=1, space="DRAM") as dpool,
        ):
            # load the full shard into SBUF as 16 [128, 768] bf16 tiles
            # (f32 -> bf16 cast happens inside the SWDGE DMA)
            xt = []
            for k in range(KT):
                xtile = xpool.tile([P, D], bf16, tag="x", name=f"x{k}")
                nc.gpsimd.dma_start(out=xtile[:], in_=x_sh[k * P : (k + 1) * P, :])
                xt.append(xtile)

            tri = accpool.tile([P, TRI_W], f32)  # packed scaled triangle

            off = 0
            for i in range(NB):
                # row-block i needs G[i-block, j-blocks j>=i] = cols 128*i..768
                c0 = P * i
                W = D - c0
                ps = pspool.tile([P, W], f32, tag=f"ps{i}", name=f"ps{i}")
                for k in range(KT):
                    lhsT = xt[k][:, P * i : P * (i + 1)]
                    for s0, s1 in _split_free(W):
                        nc.tensor.matmul(
                            ps[:, s0:s1],
                            lhsT=lhsT,
                            rhs=xt[k][:, c0 + s0 : c0 + s1],
                            start=(k == 0),
                            stop=(k == KT - 1),
                        )
                # fused (x * SCALE + BIAS) on the PSUM->SBUF copy
                nc.scalar.activation(
                    out=tri[:, off : off + W],
                    in_=ps[:],
                    func=mybir.ActivationFunctionType.Copy,
                    scale=SCALE,
                    bias=BIAS,
                )
                off += W
            assert off == TRI_W

            g_in = dpool.tile([P, TRI_W], f32, name="g_in")
            g_out = dpool.tile([ROWS_PER_CORE, TRI_W], f32, name="g_out")
            nc.sync.dma_start(out=g_in[:], in_=tri[:])
            nc.gpsimd.collective_compute(
                "ReduceScatter",
                mybir.AluOpType.add,
                replica_groups=[list(range(NCORES))],
                ins=[g_in.opt()],
                outs=[g_out.opt()],
            )
            nc.sync.dma_start(out=out_sh[:, :], in_=g_out[:])

    nc.finalize()  # Bacc: run reg-alloc + wait-legalization passes
    return nc


_NC_CACHE = None

# test-harness hooks (harness calls kernel() only; these stay defaults there)
RUN_KWARGS = {}
LAST_RESULTS = None


def _get_nc():
    global _NC_CACHE
    if _NC_CACHE is None:
        _NC_CACHE = _build()
    return _NC_CACHE


def kernel(x: np.ndarray) -> np.ndarray:
    global LAST_RESULTS
    x = np.ascontiguousarray(np.asarray(x, dtype=np.float32))
    assert x.shape == (NCORES, NSHARD, D)

    nc = _get_nc()
    in_maps = [{"x_shard": x[c]} for c in range(NCORES)]
    res = run_bass_kernel_spmd(
        nc, in_maps, core_ids=list(range(NCORES)), **RUN_KWARGS
    )
    LAST_RESULTS = res

    # gather: concatenate the per-core partition-row slices of the packed
    # triangle, then unpack the symmetric matrix (indexing only)
    packed = np.concatenate(
        [res.results[c]["out_shard"] for c in range(NCORES)], axis=0
    )  # [128, 2688]
    packed = packed.reshape(P, NTRI, P).transpose(1, 0, 2)  # [21, 128, 128]

    out = np.empty((D, D), dtype=np.float32)
    for b, (i, j) in enumerate(TRI_BLOCKS):
        blk = packed[b]
        out[P * i : P * (i + 1), P * j : P * (j + 1)] = blk
        if j != i:
            out[P * j : P * (j + 1), P * i : P * (i + 1)] = blk.T
    return out
